# revision 71
# baseline (speedup 1.0000x reference)
"""Grouped per-sample MLP (conv1d groups=B) + GroupSwish + softmax, on 8 NeuronCores.

Data-parallel over the group axis B=256: 32 groups per core, processed in
8 quads of 4 groups. Per group g: h = W1[g] @ x[g] + b1[g]; GroupSwish;
o = W2[g] @ h + b2[g]; softmax over the flattened [C*L] logits.

Key design points (fp32 baseline ~312us -> ~60us measured):
  - x and W1 ship as fp8e4m3 (13.7 MB/core total), W2/swish as bf16,
    PSUM accumulation fp32. End-to-end rel_fro error 9.3e-3 (measured,
    matches numpy simulation), under the 2e-2 gate with 2.2x margin.
  - DMA is the roofline (~350 GB/s/core measured ceiling under 8-core
    SPMD). x loads must cover all 128 partitions with long contiguous
    per-partition runs: 6 K=128 chunks per quad of 4 groups, one
    [128, 6144B-contiguous] DMA per HWDGE ring (sync+scalar) per quad.
    112-partition or 2KB-run layouts measured 25-40% slower.
  - The leftover 16 contraction rows (784 = 6*128 + 16) go to static
    zero-padded [112-row] buffers (3-deep ping-pong, SWDGE-loaded) so
    the tail matmul runs K=112 in the same PE tile mode.
  - Every matmul is K<=128, M=32 at (128,32) tile mode -> zero
    mode-switch drains. 4 groups share the PE via column tiling
    (tile_position=(0,32j), explicit because base_partition=96 cannot
    auto-derive); emission chunk-outer/group-inner so the 4 column
    strips pipeline at ~215ns per 512-col slot (warm).
  - ~20 warm-up matmuls on scratch SBUF run during the load ramp to
    push the PE HAM clock gate to 2.4 GHz before real work arrives
    (cold matmuls are ~1.7x slower and made the PE the bottleneck).
  - 3-stage software pipeline: W1+swish(q) | W2+exp(q-1) |
    softmax-normalize+store(q-2), so no PE instruction ever waits on
    the ACT/DVE swish chain (strict per-engine FIFOs + in-order PE).
  - All of W1 is preloaded in one DMA before the x stream (per-quad W1
    loads queued FIFO behind x prefetches added ~10us of stalls).
  - Activations/DVE ops run on whole [128, 512] quads (engine cost is
    per-free-dim-element, not per-partition: 4 groups for the price
    of 1). GroupSwish via tanh (only ACT table with tanh AND exp):
    x*sigmoid(sp*x) = (x/2)*(1+tanh(sp*x/2)).
  - W2 per group is a zero-padded [128, 32] column block (rows
    32j..32j+31 hold W2[g].T/1.1) -> logit rows for pad lanes compute
    as exactly 0, so exp/softmax-reduce see bounded values (no NaN/Inf
    paths anywhere; 0*Inf is unreachable by construction).
  - Softmax cross-partition sum AND its broadcast are (128,32)-mode
    matmuls against constant selector matrices, output [128,1] so the
    final scale is a plain per-partition tensor_scalar.
  - Output is stored padded [NQ, 128, L] bf16 in ONE dense DMA per
    quad (the 4x[10,512] version serialized ~1us/DMA on the SWDGE Q7);
    host slices the 10 logit rows and casts to fp32. Last two quads'
    stores ride the by-then-idle sync ring to cut the pipeline tail.
"""

import os
import numpy as np
import ml_dtypes
from contextlib import ExitStack

import concourse.mybir as mybir
import concourse.tile as tile
from concourse import bacc
from concourse.bass_utils import run_bass_kernel_spmd

B, X, Z, C, L = 256, 784, 32, 10, 512
NCORE = 8
GPC = B // NCORE  # 32 groups per core
NQ = GPC // 4  # 8 quads of 4 groups
NCH = 7  # contraction chunks
KP = X // NCH  # 112 rows per chunk
F32 = mybir.dt.float32
BF16 = mybir.dt.bfloat16
FP8 = mybir.dt.float8e4

NP_BF16 = ml_dtypes.bfloat16
NP_FP8 = ml_dtypes.float8_e4m3fn

DEFAULT_CFG = dict(
    x_bufs=8,
    s_bufs=3,
    h_bufs=3,
    o_bufs=2,
    warmup=20,
)

_CACHE: dict = {}


def _build(cfg=DEFAULT_CFG):
    nc = bacc.Bacc("TRN2", target_bir_lowering=False, debug=False)

    xm = nc.dram_tensor("xm", [NQ, 2, 128, 3 * 4 * L], FP8, kind="ExternalInput").ap()
    xtl = nc.dram_tensor("xtl", [NQ, 16, 4 * L], FP8, kind="ExternalInput").ap()
    w1m = nc.dram_tensor("w1m", [128, NQ * 4 * 6 * Z], FP8, kind="ExternalInput").ap()
    w1t = nc.dram_tensor("w1t", [KP, NQ * 4 * Z], FP8, kind="ExternalInput").ap()
    w2c = nc.dram_tensor("w2c", [128, NQ * 4 * Z], BF16, kind="ExternalInput").ap()
    onest = nc.dram_tensor("onest", [128, 4 * Z], BF16, kind="ExternalInput").ap()
    sphq = nc.dram_tensor("sphq", [128, NQ], F32, kind="ExternalInput").ap()
    spbq = nc.dram_tensor("spbq", [128, NQ], F32, kind="ExternalInput").ap()
    b1q = nc.dram_tensor("b1q", [128, NQ], F32, kind="ExternalInput").ap()
    b2q = nc.dram_tensor("b2q", [128, NQ], F32, kind="ExternalInput").ap()
    out = nc.dram_tensor("out", [NQ, 128, L], BF16, kind="ExternalOutput").ap()

    with tile.TileContext(nc) as tc, ExitStack() as ctx:
        consts = ctx.enter_context(tc.tile_pool(name="consts", bufs=1))
        xpool = ctx.enter_context(tc.tile_pool(name="x", bufs=cfg["x_bufs"]))
        spool = ctx.enter_context(tc.tile_pool(name="act", bufs=1))
        pps = ctx.enter_context(tc.tile_pool(name="pps", bufs=1, space="PSUM"))

        # statically rotated buffers (one Tile instance per buffer, reused
        # across quads): every .tile() call costs ~140ns of TileRelease
        # semaphore traffic in the NEFF epilogue, so per-quad instances
        # are replaced by rings. Dep tracking still serializes reuse.
        h4s = [pps.tile([128, L], F32, name=f"h4_{i}") for i in range(3)]
        o4s = [pps.tile([128, L], F32, name=f"o4_{i}") for i in range(2)]
        tts = [pps.tile([128, 1], F32, name=f"tt_{i}") for i in range(2)]
        ts = [spool.tile([128, L], BF16, name=f"t_{i}") for i in range(3)]
        us = [spool.tile([128, L], BF16, name=f"u_{i}") for i in range(3)]
        sws_r = [spool.tile([128, L], BF16, name=f"sw_{i}") for i in range(3)]
        expos_r = [spool.tile([128, L], BF16, name=f"ex_{i}") for i in range(3)]
        ess = [spool.tile([128, 1], F32, name=f"es_{i}") for i in range(2)]
        ebs = [spool.tile([128, 1], BF16, name=f"eb_{i}") for i in range(2)]
        ivs = [spool.tile([128, 1], F32, name=f"iv_{i}") for i in range(2)]
        rss = [spool.tile([128, L], BF16, name=f"rs_{i}") for i in range(3)]

        w2t = consts.tile([128, NQ * 4 * Z], BF16, name="w2t")
        nc.gpsimd.dma_start(w2t[:], w2c)
        ot = consts.tile([128, 4 * Z], BF16, name="ot")
        nc.gpsimd.dma_start(ot[:], onest)
        spht = consts.tile([128, NQ], F32, name="spht")
        nc.gpsimd.dma_start(spht[:], sphq)
        spbt = consts.tile([128, NQ], F32, name="spbt")
        nc.gpsimd.dma_start(spbt[:], spbq)
        b1t = consts.tile([128, NQ], F32, name="b1t")
        nc.gpsimd.dma_start(b1t[:], b1q)
        b2t = consts.tile([128, NQ], F32, name="b2t")
        nc.gpsimd.dma_start(b2t[:], b2q)

        # all of W1 up front (0.8 MB fp8): one DMA, first in the sync ring
        wall = consts.tile([128, NQ * 4 * 6 * Z], FP8, name="wall")
        nc.sync.dma_start(wall[:], w1m)
        wtail = consts.tile([128, NQ * 4 * Z], FP8, name="wtail")
        nc.scalar.dma_start(wtail[:KP, :], w1t)

        # static ping-pong buffers for the 16-row tail chunk: rows 16..111
        # stay zero so the tail matmul can run K=112 in (128,32) mode.
        tailbs = []
        for i in range(3):
            tb = consts.tile([128, 4 * L], FP8, name=f"tailb{i}")
            nc.vector.memset(tb[:], 0.0)
            tailbs.append(tb)

        # PE warm-up: ~20 dummy matmuls on uninitialized SBUF into a
        # scratch PSUM bank. No deps -> runs immediately; pushes the HAM
        # past its 4096-cycle activity window so real matmuls run at
        # 2.4 GHz instead of 1.2.
        scr = consts.tile([128, L], FP8, name="scr")
        nc.vector.memset(scr[:], 0.5)
        scw = consts.tile([128, Z], BF16, name="scw")
        nc.vector.memset(scw[:], 0.5)
        wps = ctx.enter_context(tc.tile_pool(name="wps", bufs=1, space="PSUM"))
        warm = wps.tile([Z, L], F32, name="warm")
        for i in range(cfg.get("warmup", 20)):
            nc.tensor.matmul(warm[:], scw[:], scr[:], start=True, stop=True)

        sws = {}  # q -> swish tile
        expos = {}  # q -> (expo, esb)

        def stage1(q):
            """loads + W1 matmuls + GroupSwish for quad q."""
            # main x: chunks 0-5 over all 128 partitions. DRAM layout is
            # [q, half, c', 128, (j l)]: half h holds chunks {c', c'+3}...
            # actually half h = chunks 3h..3h+2, interleaved across the two
            # HWDGE rings. The last quad is chunk-split so its matmuls can
            # chase the arriving chunks (shorter pipeline tail).
            xt = xpool.tile([128, 6 * 4 * L], FP8, tag="xt", name=f"xt{q}")
            HB = 3 * 4 * L
            nc.sync.dma_start(xt[:, :HB], xm[q, 0])
            nc.scalar.dma_start(xt[:, HB:], xm[q, 1])
            xcs = [xt[:, c * 4 * L : (c + 1) * 4 * L] for c in range(6)]
            tb = tailbs[q % 3]
            nc.gpsimd.dma_start(tb[:16, :], xtl[q])

            h4 = h4s[q % 3]
            for c in range(NCH):
                for j in range(4):
                    if c < 6:
                        lhsT = wall[:, ((q * 4 + j) * 6 + c) * Z : ((q * 4 + j) * 6 + c + 1) * Z]
                        rhs = xcs[c][:, j * L : (j + 1) * L]
                    else:
                        lhsT = wtail[:KP, (q * 4 + j) * Z : (q * 4 + j + 1) * Z]
                        rhs = tb[:KP, j * L : (j + 1) * L]
                    nc.tensor.matmul(
                        h4[32 * j : 32 * j + 32, :],
                        lhsT,
                        rhs,
                        start=(c == 0),
                        stop=(c == NCH - 1),
                        tile_position=(0, 32 * j),
                    )

            t = ts[q % 3]
            nc.scalar.activation(
                t[:],
                h4[:],
                mybir.ActivationFunctionType.Tanh,
                bias=spbt[:, q : q + 1],
                scale=spht[:, q : q + 1],
            )
            u = us[q % 3]
            nc.vector.tensor_scalar(
                u[:],
                h4[:],
                b1t[:, q : q + 1],
                0.5,
                op0=mybir.AluOpType.add,
                op1=mybir.AluOpType.mult,
            )
            sw = sws_r[q % 3]
            nc.vector.scalar_tensor_tensor(
                sw[:],
                t[:],
                1.0,
                u[:],
                op0=mybir.AluOpType.add,
                op1=mybir.AluOpType.mult,
            )
            sws[q] = sw

        def stage2(q):
            """W2 matmuls + exp for quad q (emitted one quad later)."""
            o4 = o4s[q % 2]
            sw = sws.pop(q)
            for j in range(4):
                g = 4 * q + j
                nc.tensor.matmul(
                    o4[32 * j : 32 * j + 32, :],
                    w2t[:, g * Z : (g + 1) * Z],
                    sw[:],
                    start=True,
                    stop=True,
                    tile_position=(0, 32 * j),
                )
            expo = expos_r[q % 3]
            esum = ess[q % 2]
            nc.scalar.activation(
                expo[:],
                o4[:],
                mybir.ActivationFunctionType.Exp,
                bias=b2t[:, q : q + 1],
                scale=1.0,
                accum_out=esum[:],
            )
            esb = ebs[q % 2]
            nc.vector.tensor_copy(esb[:], esum[:])
            expos[q] = (expo, esb)

        def stage3(q):
            """softmax normalization + store for quad q (two quads later)."""
            expo, esb = expos.pop(q)
            totb = tts[q % 2]
            for j in range(4):
                nc.tensor.matmul(
                    totb[32 * j : 32 * j + 32, :],
                    ot[:, 32 * j : 32 * j + 32],
                    esb[:],
                    start=True,
                    stop=True,
                    tile_position=(0, 32 * j),
                )
            invb = ivs[q % 2]
            nc.vector.reciprocal(invb[:], totb[:])
            res = rss[q % 3]
            nc.vector.tensor_scalar_mul(res[:], expo[:], invb[:])
            oe = nc.sync if q >= NQ - 2 else nc.gpsimd
            oe.dma_start(out[q], res[:])

        for q in range(NQ):
            stage1(q)
            if q >= 1:
                stage2(q - 1)
            if q >= 2:
                stage3(q - 2)
        stage2(NQ - 1)
        stage3(NQ - 2)
        stage3(NQ - 1)

    nc.compile()
    return nc


def _marshal(x, W1, b1, beta, W2, b2):
    """Full inputs -> list of per-core input dicts (all layouts hardcoded)."""
    # x: [1, B*X, L] -> [B, X, L] fp8. Main chunks c<6 (rows 128c+p), tail
    # rows 768..783. xm[q, h, p, (c', j, l)] = x[4q+j, 128*(3h+c')+p, l]
    xg = np.asarray(x, dtype=np.float32).reshape(B, X, L)
    x8f = xg.astype(NP_FP8)
    xmain = x8f[:, : 6 * 128].reshape(B // 4, 4, 2, 3, 128, L).transpose(0, 2, 4, 3, 1, 5)
    xmain = np.ascontiguousarray(xmain).reshape(B // 4, 2, 128, 3 * 4 * L)
    xtail = x8f[:, 6 * 128 :].reshape(B // 4, 4, 16, L).transpose(0, 2, 1, 3)
    xtail = np.ascontiguousarray(xtail).reshape(B // 4, 16, 4 * L)

    # W1: [B, Z, X] -> main lhsT [128, (quad, j, c<6, z)] bf16; tail
    # lhsT [KP, (quad, j, z)] with rows 16..111 zero.
    w1T = np.asarray(W1, dtype=np.float32).transpose(0, 2, 1)  # [B, X, Z]
    w1main = w1T[:, : 6 * 128].reshape(B // 4, 4, 6, 128, Z).transpose(3, 0, 1, 2, 4)
    w1main = np.ascontiguousarray(w1main).astype(NP_FP8).reshape(128, (B // 4) * 4 * 6 * Z)
    w1tail = np.zeros((KP, B, Z), dtype=NP_FP8)
    w1tail[:16] = w1T[:, 6 * 128 :].transpose(1, 0, 2).astype(NP_FP8)
    w1tail = w1tail.reshape(KP, B * Z)

    # W2 blockdiag: w2c[32j+z, g*Z+c-block] = W2[g, c, z]/1.1 (per core below)
    w2s = (np.asarray(W2, dtype=np.float32) * np.float32(1.0 / 1.1)).transpose(0, 2, 1)  # [B, Z, C]

    onest = np.zeros((128, 4 * Z), dtype=NP_BF16)
    for j in range(4):
        onest[32 * j : 32 * j + C, 32 * j : 32 * j + 32] = NP_BF16(1.0)

    b1f = np.asarray(b1, dtype=np.float32)
    b2f = np.asarray(b2, dtype=np.float32)
    spf = np.log1p(np.exp(np.asarray(beta, dtype=np.float64))).astype(np.float32)

    in_maps = []
    for core in range(NCORE):
        g0 = core * GPC
        sq = slice(core * NQ, (core + 1) * NQ)

        w2core = np.zeros((128, NQ * 4 * Z), dtype=np.float32)
        sph = np.zeros((128, NQ), dtype=np.float32)
        spb = np.zeros((128, NQ), dtype=np.float32)
        b1m = np.zeros((128, NQ), dtype=np.float32)
        b2m = np.zeros((128, NQ), dtype=np.float32)
        for q in range(NQ):
            for j in range(4):
                g = g0 + 4 * q + j
                w2core[32 * j : 32 * j + Z, (4 * q + j) * Z : (4 * q + j) * Z + C] = w2s[g]
                sph[32 * j : 32 * j + Z, q] = 0.5 * spf[g]
                spb[32 * j : 32 * j + Z, q] = 0.5 * spf[g] * b1f[g]
                b1m[32 * j : 32 * j + Z, q] = b1f[g]
                b2m[32 * j : 32 * j + C, q] = b2f[g]

        wstep = NQ * 4 * 6 * Z
        tstep = NQ * 4 * Z
        in_maps.append(
            {
                "xm": xmain[sq],
                "xtl": xtail[sq],
                "w1m": np.ascontiguousarray(
                    w1main[:, core * wstep : (core + 1) * wstep]
                ),
                "w1t": np.ascontiguousarray(
                    w1tail[:, core * tstep : (core + 1) * tstep]
                ),
                "w2c": w2core.astype(NP_BF16),
                "onest": onest,
                "sphq": sph,
                "spbq": spb,
                "b1q": b1m,
                "b2q": b2m,
            }
        )
    return in_maps


def _run(in_maps, cfg=DEFAULT_CFG, trace=False, tmpdir=None):
    key = str(sorted(cfg.items()))
    if key not in _CACHE:
        _CACHE[key] = _build(cfg)
    return run_bass_kernel_spmd(
        _CACHE[key],
        in_maps,
        core_ids=list(range(NCORE)),
        trace=trace,
        tmpdir=tmpdir,
    )


_LAST = {}


def kernel(x, W1, b1, beta, W2, b2):
    in_maps = _marshal(x, W1, b1, beta, W2, b2)
    trace = bool(os.environ.get("KERNEL_TRACE"))
    r = _run(in_maps, trace=trace, tmpdir=os.environ.get("KERNEL_TRACE_DIR"))
    _LAST["results"] = r
    outs = [
        np.ascontiguousarray(
            r.results[c]["out"].reshape(NQ, 4, 32, L)[:, :, :C, :]
        )
        .astype(np.float32)
        .reshape(GPC, C * L)
        for c in range(NCORE)
    ]
    return np.concatenate(outs, axis=0)
